# revision 2
# baseline (speedup 1.0000x reference)
# GAT (2-layer, 8-head) Trainium2 Bass kernel — v2 (factored-exp scores).
# Data-parallel over batch across 8 NeuronCores (2 batches/core).
#
# Score restructuring: exp(leaky(z)) with z = s1_i + s2_j decomposes as
#   exp(leaky(z)) = max(exp(z), exp(0.2 z))          [leaky(z) = max(z, .2z)]
# and any per-i positive factor cancels in the softmax, so dividing by
# exp(s1_i) gives the weights
#   u[j,i] = max(g_i * f2_j, e2_j),   g = exp(-.8 s1), e2 = exp(s2),
#                                     f2 = exp(.2 s2)
# which needs NO transcendentals over the S x S tensor: one fast DVE
# tensor_scalar (mult+max with two per-partition f32 scalars) per j-tile,
# then one fast tensor_tensor multiply by the shared binary keep-mask K.
# For ACT-offloaded heads the identity  u = e2 + relu(g*f2 - e2)  routes the
# 2D work through the ACT engine (relu with per-partition scale/bias), the
# e2 term going through an extra PE matmul with the shared K as stationary.
# Masks are built multiplicative ((sm==0)*adj) and transposed via DMA xbar.
import os
import numpy as np
from contextlib import ExitStack

LN_EPS = 1e-5

_CACHE = {}
LAST_EXEC_NS = None
# heads 0..NR-1 of each (batch,layer) take the ACT relu path
NR_DEFAULT = int(os.environ.get("GAT_NR", "3"))
# of the remaining heads, the last NZ route their mask-multiply to Pool
NZ_DEFAULT = int(os.environ.get("GAT_NZ", "1"))


def _bcast_ap(ap, p=128):
    """Replicate a [free...] AP across p partitions (stride-0 partition dim)."""
    import concourse.bass as bass
    return bass.AP(tensor=ap.tensor, offset=ap.offset, ap=[[0, p]] + list(ap.ap))


def _build(B2, S, M, H, L, semantic, apply_g, reps=1, NR=None, NZ=None):
    import concourse.bass as bass
    import concourse.bacc as bacc
    import concourse.tile as tile
    from concourse import mybir
    from concourse._compat import axon_active

    f16 = mybir.dt.float16
    f32 = mybir.dt.float32
    Alu = mybir.AluOpType
    Act = mybir.ActivationFunctionType

    if NR is None:
        NR = NR_DEFAULT
    if NZ is None:
        NZ = NZ_DEFAULT

    DK = M // H
    ST = S // 128          # row tiles (also column tiles)
    KT = M // 128          # contraction tiles for the projection
    HC = H * 35            # packed cols/head: 32 P + 1 one + 1 s2 + 1 s1
    nc = bacc.Bacc(
        "TRN2", target_bir_lowering=False, debug=not axon_active(), num_devices=8)
    adj_d = nc.declare_dram_parameter("adj", [B2, S, S], mybir.dt.int32, isOutput=False)
    sm_d = nc.declare_dram_parameter("smask", [B2, S, S], mybir.dt.uint8, isOutput=False)
    x0_d = nc.declare_dram_parameter("x0", [B2, S, M], f32, isOutput=False)
    pw_d = nc.declare_dram_parameter("pwcat", [L, KT, 128, HC], f16, isOutput=False)
    bc_d = nc.declare_dram_parameter("biascat", [L, HC], f32, isOutput=False)
    if apply_g:
        g_d = nc.declare_dram_parameter("lng", [L, M], f32, isOutput=False)
        b_d = nc.declare_dram_parameter("lnb", [L, M], f32, isOutput=False)
    out_d = nc.declare_dram_parameter("out", [B2, S, M], f32, isOutput=True)

    with tile.TileContext(nc) as tc, ExitStack() as ctx:
        singles = ctx.enter_context(tc.tile_pool(name="singles", bufs=1))
        persist = ctx.enter_context(tc.tile_pool(name="persist", bufs=1))
        io = ctx.enter_context(tc.tile_pool(name="io", bufs=2))
        maskw = ctx.enter_context(tc.tile_pool(name="maskw", bufs=2))
        upool = ctx.enter_context(tc.tile_pool(name="upool", bufs=2))
        vpool = ctx.enter_context(tc.tile_pool(name="vpool", bufs=2))
        gbp = ctx.enter_context(tc.tile_pool(name="gbp", bufs=4))
        xpool = ctx.enter_context(tc.tile_pool(name="xpool", bufs=4))
        lay = ctx.enter_context(tc.tile_pool(name="lay", bufs=2))
        small = ctx.enter_context(tc.tile_pool(name="small", bufs=4))
        pprojp = ctx.enter_context(tc.tile_pool(name="pprojp", bufs=2, space="PSUM"))
        dramp = ctx.enter_context(tc.tile_pool(name="dramp", bufs=2, space="DRAM"))
        pavp = ctx.enter_context(tc.tile_pool(name="pavp", bufs=4, space="PSUM"))

        eps_t = singles.tile([128, 1], f32)
        nc.vector.memset(eps_t[:], LN_EPS)

        rep_cm = tc.For_i(
            0, reps, 1, name="rep",
            hint_engines=(mybir.EngineType.PE, mybir.EngineType.DVE,
                          mybir.EngineType.Activation, mybir.EngineType.SP,
                          mybir.EngineType.Pool)) if reps > 1 else None
        if rep_cm is not None:
            ctx.enter_context(rep_cm)

        # ---------------- Stage A: combined transposed keep masks ----------
        # K[b][:, jt, i] = 1 if (adj[b, i, j]!=0 and not smask[b, i, j]) else 0
        # (j = jt*128 + partition). Built untransposed on DVE then moved
        # through the DMA xbar per 128x128 block. Optionally an smask-only
        # variant for semantic layers > 0.
        kts = []
        for b in range(B2):
            variants = [(True, persist.tile([128, ST, S], f16, tag=f"kt{b}", name=f"kt{b}"))]
            if semantic:
                variants.append((False, persist.tile([128, ST, S], f16, tag=f"ktsm{b}", name=f"ktsm{b}")))
            kts.append(variants)
            for use_adj, kt in variants:
                for s in range(ST):
                    sm_t = io.tile([128, S], mybir.dt.uint8, tag="smt")
                    nc.sync.dma_start(out=sm_t[:], in_=sm_d[b, s * 128:(s + 1) * 128, :])
                    ms = maskw.tile([128, S], f16, tag="ms")
                    if use_adj:
                        adj_t = io.tile([128, S], mybir.dt.int32, tag="adjt")
                        nc.sync.dma_start(out=adj_t[:], in_=adj_d[b, s * 128:(s + 1) * 128, :])
                        # keep = (sm == 0) * adj
                        nc.vector.scalar_tensor_tensor(
                            out=ms[:], in0=sm_t[:], scalar=0.0, in1=adj_t[:],
                            op0=Alu.is_equal, op1=Alu.mult)
                    else:
                        # keep = (sm == 0)
                        nc.vector.tensor_scalar(
                            out=ms[:], in0=sm_t[:], scalar1=0.0, scalar2=None,
                            op0=Alu.is_equal)
                    for jt in range(ST):
                        nc.sync.dma_start_transpose(
                            out=kt[:, jt, s * 128:(s + 1) * 128],
                            in_=ms[:, jt * 128:(jt + 1) * 128])

        # ---------------- x0 load & cast ----------------
        xf16 = {}
        for b in range(B2):
            xf16[(b, 0)] = xpool.tile([128, ST, M], f16, tag="xf16", name=f"xf16_{b}_0")
            for s in range(ST):
                xs = io.tile([128, M], f32, tag="x0s")
                nc.sync.dma_start(out=xs[:], in_=x0_d[b, s * 128:(s + 1) * 128, :])
                nc.vector.tensor_copy(out=xf16[(b, 0)][:, s, :], in_=xs[:])

        # ---------------- Layers ----------------
        for l in range(L):
            pw_sb = [lay.tile([128, HC], f16, tag="pwsb", name=f"pwsb{_}") for _ in range(KT)]
            for kt_i in range(KT):
                nc.sync.dma_start(out=pw_sb[kt_i][:], in_=pw_d[l, kt_i])
            biasb = lay.tile([128, HC], f32, tag="biasb")
            nc.sync.dma_start(out=biasb[:], in_=_bcast_ap(bc_d[l]))
            if apply_g:
                gb_ln = lay.tile([128, M], f32, tag="gbln")
                nc.sync.dma_start(out=gb_ln[:], in_=_bcast_ap(g_d[l]))
                bb_ln = lay.tile([128, M], f32, tag="bbln")
                nc.sync.dma_start(out=bb_ln[:], in_=_bcast_ap(b_d[l]))

            for b in range(B2):
                x16 = xf16[(b, l)]
                kt = kts[b][1][1] if (semantic and l > 0) else kts[b][0][1]

                # xT (f16, [m, s] layout) via DMA xbar transposes
                xT = lay.tile([128, KT, S], f16, tag="xT")
                for kt_i in range(KT):
                    for s in range(ST):
                        nc.sync.dma_start_transpose(
                            out=xT[:, kt_i, s * 128:(s + 1) * 128],
                            in_=x16[:, s, kt_i * 128:(kt_i + 1) * 128])

                # Projection: P_sb[:, s, h, 0:32] = P, [...,32] = 1.0,
                # [...,33] = s2, [...,34] = s1 (+attn bias)
                P_sb = lay.tile([128, ST, H, 35], f16, tag="Psb")
                for s in range(ST):
                    pproj = pprojp.tile([128, HC], f32, tag="pproj")
                    for kt_i in range(KT):
                        nc.tensor.matmul(
                            pproj[:], xT[:, kt_i, s * 128:(s + 1) * 128], pw_sb[kt_i][:],
                            start=(kt_i == 0), stop=(kt_i == KT - 1))
                    nc.vector.scalar_tensor_tensor(
                        out=P_sb[:, s, :, :], in0=pproj[:], scalar=0.0, in1=biasb[:],
                        op0=Alu.add, op1=Alu.add)

                # Tiny per-(j or i) exponentials.
                # e2/f2 (f32, used as tensor_scalar scalars and ACT
                # scale/bias), g (f16, bounced to DRAM for broadcast).
                e2f = small.tile([128, ST, H], f32, tag="e2f")
                nc.scalar.activation(out=e2f[:], in_=P_sb[:, :, :, 33], func=Act.Exp)
                f2f = small.tile([128, ST, H], f32, tag="f2f")
                nc.scalar.activation(out=f2f[:], in_=P_sb[:, :, :, 33], func=Act.Exp,
                                     scale=0.2)
                g16 = small.tile([128, ST, H], f16, tag="g16")
                nc.scalar.activation(out=g16[:], in_=P_sb[:, :, :, 34], func=Act.Exp,
                                     scale=-0.8)
                if NR > 0:
                    ne2 = small.tile([128, ST, H], f32, tag="ne2")
                    nc.vector.tensor_scalar(
                        out=ne2[:], in0=e2f[:], scalar1=-1.0, scalar2=None,
                        op0=Alu.mult)

                # bounce g to DRAM in [h, s] row-major layout for broadcasts
                gdram = dramp.tile([H, S], f16, tag="gdram")
                for st in range(ST):
                    nc.sync.dma_start(
                        out=bass.AP(tensor=gdram.tensor, offset=gdram.offset + st * 128,
                                    ap=[[1, 128], [S, H]]),
                        in_=g16[:, st, :])

                conc = lay.tile([128, ST, M], f16, tag="conc")
                for h in range(H):
                    gb = gbp.tile([128, S], f16, tag="gb")
                    nc.sync.dma_start(out=gb[:], in_=_bcast_ap(gdram[h, :]))
                    pav = pavp.tile([128, ST, 36], f32, tag="pav")
                    mask_on_pool = (h >= H - NZ)
                    if h < NR:
                        # ACT path: r = relu(g_i * f2_j - e2_j); u = e2 + r.
                        # e2 term via shared-K matmul against P2 = e2 * P.
                        P2h = small.tile([128, ST, 33], f16, tag="P2h")
                        r1 = upool.tile([128, ST, S], f16, tag="u")
                        for jt in range(ST):
                            nc.vector.tensor_scalar(
                                out=P2h[:, jt, :], in0=P_sb[:, jt, h, 0:33],
                                scalar1=e2f[:, jt, h:h + 1], scalar2=None,
                                op0=Alu.mult)
                            nc.scalar.activation(
                                out=r1[:, jt, :], in_=gb[:], func=Act.Relu,
                                bias=ne2[:, jt, h:h + 1], scale=f2f[:, jt, h:h + 1])
                        v = vpool.tile([128, ST, S], f16, tag="v")
                        eng = nc.gpsimd if mask_on_pool else nc.vector
                        eng.tensor_tensor(out=v[:], in0=r1[:], in1=kt[:], op=Alu.mult)
                        for ib in range(ST):
                            for jt in range(ST):
                                nc.tensor.matmul(
                                    pav[:, ib, 0:33],
                                    kt[:, jt, ib * 128:(ib + 1) * 128],
                                    P2h[:, jt, :],
                                    start=(jt == 0), stop=False)
                            for jt in range(ST):
                                nc.tensor.matmul(
                                    pav[:, ib, 0:33],
                                    v[:, jt, ib * 128:(ib + 1) * 128],
                                    P_sb[:, jt, h, 0:33],
                                    start=False, stop=(jt == ST - 1))
                    else:
                        # DVE path: u = max(g_i * f2_j, e2_j), v = u * K
                        u = upool.tile([128, ST, S], f16, tag="u")
                        for jt in range(ST):
                            nc.vector.tensor_scalar(
                                out=u[:, jt, :], in0=gb[:],
                                scalar1=f2f[:, jt, h:h + 1],
                                scalar2=e2f[:, jt, h:h + 1],
                                op0=Alu.mult, op1=Alu.max)
                        v = vpool.tile([128, ST, S], f16, tag="v")
                        eng = nc.gpsimd if mask_on_pool else nc.vector
                        eng.tensor_tensor(out=v[:], in0=u[:], in1=kt[:], op=Alu.mult)
                        for ib in range(ST):
                            for jt in range(ST):
                                nc.tensor.matmul(
                                    pav[:, ib, 0:33],
                                    v[:, jt, ib * 128:(ib + 1) * 128],
                                    P_sb[:, jt, h, 0:33],
                                    start=(jt == 0), stop=(jt == ST - 1))
                    rec = small.tile([128, ST], f32, tag="rec")
                    nc.vector.reciprocal(out=rec[:], in_=pav[:, :, 32])
                    nc.vector.tensor_tensor(
                        out=conc[:, :, h * DK:(h + 1) * DK],
                        in0=pav[:, :, 0:DK],
                        in1=rec[:].rearrange("p (s one) -> p s one", one=1).broadcast_to([128, ST, DK]),
                        op=Alu.mult)

                # Residual + LayerNorm
                rr = lay.tile([128, ST, M], f16, tag="rr")
                sums = small.tile([128, ST], f32, tag="sums")
                sq = small.tile([128, ST], f32, tag="sq")
                for s in range(ST):
                    nc.vector.scalar_tensor_tensor(
                        out=rr[:, s, :], in0=conc[:, s, :], scalar=0.0, in1=x16[:, s, :],
                        op0=Alu.add, op1=Alu.add, accum_out=sums[:, s:s + 1])
                    scr = small.tile([128, M], f32, tag="scr")
                    nc.scalar.activation(out=scr[:], in_=rr[:, s, :], func=Act.Square,
                                         accum_out=sq[:, s:s + 1])
                mu = small.tile([128, ST], f32, tag="mu")
                nc.vector.tensor_scalar(out=mu[:], in0=sums[:], scalar1=1.0 / M,
                                        scalar2=None, op0=Alu.mult)
                mu2 = small.tile([128, ST], f32, tag="mu2")
                nc.vector.tensor_tensor(out=mu2[:], in0=mu[:], in1=mu[:], op=Alu.mult)
                var = small.tile([128, ST], f32, tag="var")
                nc.vector.scalar_tensor_tensor(
                    out=var[:], in0=sq[:], scalar=1.0 / M, in1=mu2[:],
                    op0=Alu.mult, op1=Alu.subtract)
                # rstd = 1/sqrt(var+eps) via Babylonian iterations + reciprocal
                # (avoids Sqrt/Ln ACT table switches away from the exp set)
                ve = small.tile([128, ST], f32, tag="ve")
                nc.vector.tensor_scalar(out=ve[:], in0=var[:], scalar1=LN_EPS,
                                        scalar2=None, op0=Alu.add)
                std = small.tile([128, ST], f32, tag="std")
                nc.vector.tensor_scalar(out=std[:], in0=ve[:], scalar1=0.4,
                                        scalar2=0.7, op0=Alu.mult, op1=Alu.add)
                for _it in range(3):
                    rs = small.tile([128, ST], f32, tag="rs", name=f"rs{_it}")
                    nc.vector.reciprocal(out=rs[:], in_=std[:])
                    tdiv = small.tile([128, ST], f32, tag="tdiv", name=f"tdiv{_it}")
                    nc.vector.tensor_tensor(out=tdiv[:], in0=ve[:], in1=rs[:],
                                            op=Alu.mult)
                    usum = small.tile([128, ST], f32, tag="usum", name=f"usum{_it}")
                    nc.vector.tensor_tensor(out=usum[:], in0=std[:], in1=tdiv[:],
                                            op=Alu.add)
                    std2 = small.tile([128, ST], f32, tag="std", name=f"std{_it}")
                    nc.vector.tensor_scalar(out=std2[:], in0=usum[:], scalar1=0.5,
                                            scalar2=None, op0=Alu.mult)
                    std = std2
                rstd = small.tile([128, ST], f32, tag="rstd")
                nc.vector.reciprocal(out=rstd[:], in_=std[:])

                last = (l == L - 1)
                if last:
                    y32 = lay.tile([128, ST, M], f32, tag="y32")
                else:
                    xf16[(b, l + 1)] = xpool.tile([128, ST, M], f16, tag="xf16", name=f"xf16_{b}_{l+1}")
                for s in range(ST):
                    if apply_g:
                        tmp = small.tile([128, M], f32, tag="ytmp")
                        nc.vector.tensor_scalar(
                            out=tmp[:], in0=rr[:, s, :], scalar1=mu[:, s:s + 1],
                            scalar2=rstd[:, s:s + 1], op0=Alu.subtract, op1=Alu.mult)
                        tmp2 = small.tile([128, M], f32, tag="ytmp2")
                        nc.vector.tensor_tensor(out=tmp2[:], in0=tmp[:], in1=gb_ln[:], op=Alu.mult)
                        ydst = y32[:, s, :] if last else xf16[(b, l + 1)][:, s, :]
                        nc.vector.tensor_tensor(out=ydst, in0=tmp2[:], in1=bb_ln[:], op=Alu.add)
                    else:
                        ydst = y32[:, s, :] if last else xf16[(b, l + 1)][:, s, :]
                        nc.vector.tensor_scalar(
                            out=ydst, in0=rr[:, s, :], scalar1=mu[:, s:s + 1],
                            scalar2=rstd[:, s:s + 1], op0=Alu.subtract, op1=Alu.mult)
                if last:
                    nc.sync.dma_start(
                        out=out_d[b].rearrange("(s p) m -> p s m", p=128), in_=y32[:])
    nc.compile()
    return nc


def _get_nc(key):
    if key not in _CACHE:
        _CACHE[key] = _build(*key)
    return _CACHE[key]


def _pack_weights(proj_w, proj_b, attn_w, attn_b):
    L, H, M, DK = proj_w.shape
    KT = M // 128
    HC = H * 35
    pwcat = np.zeros((L, M, H, 35), np.float32)
    biascat = np.zeros((L, H, 35), np.float32)
    for l in range(L):
        a1, a2 = attn_w[l, :DK], attn_w[l, DK:]
        for h in range(H):
            pwcat[l, :, h, :32] = proj_w[l, h]
            pwcat[l, :, h, 33] = proj_w[l, h] @ a2
            pwcat[l, :, h, 34] = proj_w[l, h] @ a1
            biascat[l, h, :32] = proj_b[l, h]
            biascat[l, h, 32] = 1.0
            biascat[l, h, 33] = proj_b[l, h] @ a2
            biascat[l, h, 34] = proj_b[l, h] @ a1 + attn_b[l]
    return (pwcat.reshape(L, KT, 128, HC).astype(np.float16),
            biascat.reshape(L, HC))


def _prepare(adj, inputs, score_mask, type, proj_w, proj_b, attn_w, attn_b, ln_g, ln_b):
    adj = np.asarray(adj)
    inputs = np.asarray(inputs, dtype=np.float32)
    score_mask = np.asarray(score_mask)
    proj_w = np.asarray(proj_w, dtype=np.float32)
    proj_b = np.asarray(proj_b, dtype=np.float32)
    attn_w = np.asarray(attn_w, dtype=np.float32)
    attn_b = np.asarray(attn_b, dtype=np.float32)
    ln_g = np.asarray(ln_g, dtype=np.float32)
    ln_b = np.asarray(ln_b, dtype=np.float32)

    B, S, M = inputs.shape
    L, H = proj_w.shape[0], proj_w.shape[1]
    NCORES = 8
    B2 = B // NCORES
    semantic = bool(np.asarray(type) == 1)
    apply_g = not (np.allclose(ln_g, 1.0) and np.allclose(ln_b, 0.0))

    pwcat, biascat = _pack_weights(proj_w, proj_b, attn_w, attn_b)
    sm_u8 = np.ascontiguousarray(score_mask[:, 0]).astype(np.uint8)
    adj_i32 = np.ascontiguousarray(adj.astype(np.int32))

    in_maps = []
    for c in range(NCORES):
        m = {
            "adj": adj_i32[c * B2:(c + 1) * B2],
            "smask": sm_u8[c * B2:(c + 1) * B2],
            "x0": np.ascontiguousarray(inputs[c * B2:(c + 1) * B2]),
            "pwcat": pwcat, "biascat": biascat,
        }
        if apply_g:
            m["lng"] = ln_g
            m["lnb"] = ln_b
        in_maps.append(m)

    return (B2, S, M, H, L, semantic, apply_g), in_maps


def kernel(**inputs):
    from concourse.bass_utils import run_bass_kernel_spmd
    key, in_maps = _prepare(**inputs)
    nc = _get_nc(key)
    res = run_bass_kernel_spmd(nc, in_maps, core_ids=list(range(len(in_maps))),
                               trace=bool(int(os.environ.get("GAT_TRACE", "0"))))
    global LAST_EXEC_NS
    LAST_EXEC_NS = res.exec_time_ns
    out = np.concatenate([r["out"] for r in res.results], axis=0)
    return out.astype(np.float32)


def measure_hw_s(reps=64, n_runs=3, **inputs):
    """Estimate per-iteration device time by timing a reps-looped variant
    against the reps=1 variant (amortizes axon dispatch + transfer)."""
    import time
    from concourse.bass_utils import run_bass_kernel_spmd
    key, in_maps = _prepare(**inputs)
    cores = list(range(len(in_maps)))
    nc1 = _get_nc(key)
    ncR = _build(*key, reps=reps)

    def timed(nc):
        best = None
        for _ in range(n_runs):
            t0 = time.time()
            run_bass_kernel_spmd(nc, in_maps, core_ids=cores)
            dt = time.time() - t0
            best = dt if best is None else min(best, dt)
        return best

    t1 = timed(nc1)
    tR = timed(ncR)
    per_iter = (tR - t1) / (reps - 1)
    return per_iter, t1, tR


# revision 11
# speedup vs baseline: 1.1870x; 1.1870x over previous
# GAT (2-layer, 8-head) Trainium2 Bass kernel — v3 (factored-exp scores,
# batched DMA, dual DGE queues).
# Data-parallel over batch across 8 NeuronCores (2 batches/core).
#
# Score restructuring: exp(leaky(z)) with z = s1_i + s2_j decomposes as
#   exp(leaky(z)) = max(exp(z), exp(0.2 z))          [leaky(z) = max(z, .2z)]
# and any per-i positive factor cancels in the softmax, so dividing by
# exp(s1_i) gives the weights
#   u[j,i] = max(g_i * f2_j, e2_j),   g = exp(-.8 s1), e2 = exp(s2),
#                                     f2 = exp(.2 s2)
# which needs NO transcendentals over the S x S tensor: one fast DVE
# tensor_scalar (mult+max, two per-partition f32 scalars) per j-tile, then
# one fast tensor_tensor multiply with the shared binary keep mask K.
# For ACT-offloaded heads, u = e2 + relu(g*f2 - e2) routes the 2D work
# through ACT (relu with per-partition scale/bias); the e2 term rides an
# extra PE matmul with the shared K as stationary operand.
#
# DMA discipline: everything batched (chunked input loads, one xbar
# transpose per 128-row group, whole-slab bounces/broadcasts) and spread
# across both HW DGE queues (SP via nc.sync, Activation via nc.scalar).
import os
import numpy as np
from contextlib import ExitStack

LN_EPS = 1e-5

_CACHE = {}
LAST_EXEC_NS = None
# heads 0..NR-1 of each (batch,layer) take the ACT relu path
NR_DEFAULT = int(os.environ.get("GAT_NR", "3"))
# of the remaining heads, the last NZ route their mask-multiply to Pool
NZ_DEFAULT = int(os.environ.get("GAT_NZ", "1"))
ONE_QUEUE = bool(int(os.environ.get("GAT_1Q", "0")))


def _bcast_ap(ap, p=128):
    """Replicate a [free...] AP across p partitions (stride-0 partition dim)."""
    import concourse.bass as bass
    return bass.AP(tensor=ap.tensor, offset=ap.offset, ap=[[0, p]] + list(ap.ap))


def _build(B2, S, M, H, L, semantic, apply_g, reps=1, NR=None, NZ=None):
    import concourse.bass as bass
    import concourse.bacc as bacc
    import concourse.tile as tile
    from concourse import mybir
    from concourse._compat import axon_active

    f16 = mybir.dt.float16
    f32 = mybir.dt.float32
    Alu = mybir.AluOpType
    Act = mybir.ActivationFunctionType

    if NR is None:
        NR = NR_DEFAULT
    if NZ is None:
        NZ = NZ_DEFAULT

    DK = M // H
    ST = S // 128          # row tiles (also column tiles)
    KT = M // 128          # contraction tiles for the projection
    HC = H * 35            # packed cols/head: 32 P + 1 one + 1 s2 + 1 s1
    nc = bacc.Bacc(
        "TRN2", target_bir_lowering=False, debug=not axon_active(), num_devices=8)
    adj_d = nc.declare_dram_parameter("adj", [B2, S, S], mybir.dt.int32, isOutput=False)
    sm_d = nc.declare_dram_parameter("smask", [B2, S, S], mybir.dt.uint8, isOutput=False)
    x0_d = nc.declare_dram_parameter("x0", [B2, S, M], f32, isOutput=False)
    pw_d = nc.declare_dram_parameter("pwcat", [L, KT, 128, HC], f16, isOutput=False)
    bc_d = nc.declare_dram_parameter("biascat", [L, HC], f32, isOutput=False)
    if apply_g:
        g_d = nc.declare_dram_parameter("lng", [L, M], f32, isOutput=False)
        b_d = nc.declare_dram_parameter("lnb", [L, M], f32, isOutput=False)
    out_d = nc.declare_dram_parameter("out", [B2, S, M], f32, isOutput=True)

    dma_b = nc.sync if ONE_QUEUE else nc.scalar
    with tile.TileContext(nc) as tc, ExitStack() as ctx:
        persist = ctx.enter_context(tc.tile_pool(name="persist", bufs=1))
        io = ctx.enter_context(tc.tile_pool(name="io", bufs=2))
        maskw = ctx.enter_context(tc.tile_pool(name="maskw", bufs=2))
        uvpool = ctx.enter_context(tc.tile_pool(name="uvpool", bufs=3))
        gbp = ctx.enter_context(tc.tile_pool(name="gbp", bufs=2))
        xpool = ctx.enter_context(tc.tile_pool(name="xpool", bufs=4))
        lay = ctx.enter_context(tc.tile_pool(name="lay", bufs=2))
        ypool = ctx.enter_context(tc.tile_pool(name="ypool", bufs=1))
        small = ctx.enter_context(tc.tile_pool(name="small", bufs=3))
        pprojp = ctx.enter_context(tc.tile_pool(name="pprojp", bufs=2, space="PSUM"))
        dramp = ctx.enter_context(tc.tile_pool(name="dramp", bufs=2, space="DRAM"))
        pavp = ctx.enter_context(tc.tile_pool(name="pavp", bufs=4, space="PSUM"))

        rep_cm = tc.For_i(
            0, reps, 1, name="rep",
            hint_engines=(mybir.EngineType.PE, mybir.EngineType.DVE,
                          mybir.EngineType.Activation, mybir.EngineType.SP,
                          mybir.EngineType.Pool)) if reps > 1 else None
        if rep_cm is not None:
            ctx.enter_context(rep_cm)

        # ---------------- Stage A: combined transposed keep masks ----------
        # ktg[b][jp, s, jt, ip] = keep at (i = s*128+ip, j = jt*128+jp):
        # 1 if (adj!=0 and not smask) else 0. Built untransposed on DVE in
        # [i, j] layout, then one batched DMA xbar transpose per s row-tile.
        CH2 = 2  # s-tiles per input-load chunk
        kts = []
        for b in range(B2):
            variants = [(True, persist.tile([128, ST, ST, 128], f16, tag=f"kt{b}", name=f"kt{b}"))]
            if semantic:
                variants.append((False, persist.tile([128, ST, ST, 128], f16, tag=f"ktsm{b}", name=f"ktsm{b}")))
            kts.append(variants)
            for use_adj, ktg in variants:
                for c in range(ST // CH2):
                    sm_t = io.tile([128, CH2, S], mybir.dt.uint8, tag="smt")
                    dma_b.dma_start(
                        out=sm_t[:],
                        in_=sm_d[b, c * CH2 * 128:(c + 1) * CH2 * 128, :].rearrange(
                            "(k p) j -> p k j", p=128))
                    if use_adj:
                        adj_t = io.tile([128, CH2, S], mybir.dt.int32, tag="adjt")
                        nc.sync.dma_start(
                            out=adj_t[:],
                            in_=adj_d[b, c * CH2 * 128:(c + 1) * CH2 * 128, :].rearrange(
                                "(k p) j -> p k j", p=128))
                    for k in range(CH2):
                        s = c * CH2 + k
                        ms = maskw.tile([128, S], f16, tag="ms")
                        if use_adj:
                            # keep = (sm == 0) * adj
                            nc.vector.scalar_tensor_tensor(
                                out=ms[:], in0=sm_t[:, k, :], scalar=0.0,
                                in1=adj_t[:, k, :],
                                op0=Alu.is_equal, op1=Alu.mult)
                        else:
                            nc.vector.tensor_scalar(
                                out=ms[:], in0=sm_t[:, k, :], scalar1=0.0,
                                scalar2=None, op0=Alu.is_equal)
                        eng = nc.sync if (s % 2 == 0) else dma_b
                        eng.dma_start_transpose(out=ktg[:, s, :, :], in_=ms[:])

        # ---------------- x0 load & cast ----------------
        CH4 = 2
        xf16 = {}
        for b in range(B2):
            xf16[(b, 0)] = xpool.tile([128, ST, M], f16, tag="xf16", name=f"xf16_{b}_0")
            for c in range(ST // CH4):
                xs = io.tile([128, CH4, M], f32, tag="x0s")
                dma_b.dma_start(
                    out=xs[:],
                    in_=x0_d[b, c * CH4 * 128:(c + 1) * CH4 * 128, :].rearrange(
                        "(k p) m -> p k m", p=128))
                for k in range(CH4):
                    nc.vector.tensor_copy(out=xf16[(b, 0)][:, c * CH4 + k, :],
                                          in_=xs[:, k, :])

        # ---------------- Layers ----------------
        for l in range(L):
            pw_sb = [lay.tile([128, HC], f16, tag="pwsb", name=f"pwsb{_}") for _ in range(KT)]
            for kt_i in range(KT):
                nc.sync.dma_start(out=pw_sb[kt_i][:], in_=pw_d[l, kt_i])
            biasb = lay.tile([128, HC], f32, tag="biasb")
            nc.sync.dma_start(out=biasb[:], in_=_bcast_ap(bc_d[l]))
            if apply_g:
                gb_ln = lay.tile([128, M], f32, tag="gbln")
                nc.sync.dma_start(out=gb_ln[:], in_=_bcast_ap(g_d[l]))
                bb_ln = lay.tile([128, M], f32, tag="bbln")
                nc.sync.dma_start(out=bb_ln[:], in_=_bcast_ap(b_d[l]))

            for b in range(B2):
                x16 = xf16[(b, l)]
                ktg = kts[b][1][1] if (semantic and l > 0) else kts[b][0][1]

                # xT via one batched DMA xbar transpose:
                # xTn[p, s*KT+kt, i] = x16[i, s, kt*128+p]
                xTn = lay.tile([128, ST * KT, 128], f16, tag="xTn")
                nc.sync.dma_start_transpose(
                    out=xTn[:], in_=x16[:].rearrange("p s m -> p (s m)"))

                # Projection: P_sb[:, s, h, 0:32] = P, [...,32] = 1.0,
                # [...,33] = s2, [...,34] = s1 (+attn bias)
                P_sb = lay.tile([128, ST, H, 35], f16, tag="Psb")
                for s in range(ST):
                    pproj = pprojp.tile([128, HC], f32, tag="pproj")
                    for kt_i in range(KT):
                        nc.tensor.matmul(
                            pproj[:], xTn[:, s * KT + kt_i, :], pw_sb[kt_i][:],
                            start=(kt_i == 0), stop=(kt_i == KT - 1))
                    nc.vector.scalar_tensor_tensor(
                        out=P_sb[:, s, :, :], in0=pproj[:], scalar=0.0, in1=biasb[:],
                        op0=Alu.add, op1=Alu.add)

                # Tiny per-(j or i) exponentials.
                e2f = small.tile([128, ST, H], f32, tag="e2f")
                nc.scalar.activation(out=e2f[:], in_=P_sb[:, :, :, 33], func=Act.Exp)
                f2f = small.tile([128, ST, H], f32, tag="f2f")
                nc.scalar.activation(out=f2f[:], in_=P_sb[:, :, :, 33], func=Act.Exp,
                                     scale=0.2)
                g16 = small.tile([128, ST, H], f16, tag="g16")
                nc.scalar.activation(out=g16[:], in_=P_sb[:, :, :, 34], func=Act.Exp,
                                     scale=-0.8)
                if NR > 0:
                    ne2 = small.tile([128, ST, H], f32, tag="ne2")
                    nc.vector.tensor_scalar(
                        out=ne2[:], in0=e2f[:], scalar1=-1.0, scalar2=None,
                        op0=Alu.mult)

                # bounce g to DRAM in [h, s] row-major layout for broadcasts
                gdram = dramp.tile([H, S], f16, tag="gdram")
                for st in range(ST):
                    dma_b.dma_start(
                        out=bass.AP(tensor=gdram.tensor, offset=gdram.offset + st * 128,
                                    ap=[[1, 128], [S, H]]),
                        in_=g16[:, st, :])

                # broadcast g rows into two [128, H/2, S] half tiles
                ghalf = []
                for hh in range(2):
                    gt = gbp.tile([128, H // 2, S], f16, tag="gb4", name=f"gb4_{hh}")
                    dma_b.dma_start(
                        out=gt[:],
                        in_=_bcast_ap(gdram[hh * (H // 2):(hh + 1) * (H // 2), :]))
                    ghalf.append(gt)

                conc = lay.tile([128, ST, M], f16, tag="conc")
                for h in range(H):
                    gb = ghalf[h // (H // 2)][:, h % (H // 2), :]
                    pav = pavp.tile([128, ST, 36], f32, tag="pav")
                    mask_on_pool = (h >= H - NZ)
                    if h < NR:
                        # ACT path: r = relu(g_i * f2_j - e2_j); u = e2 + r.
                        P2h = small.tile([128, ST, 33], f16, tag="P2h")
                        r1 = uvpool.tile([128, ST, S], f16, tag="uv")
                        for jt in range(ST):
                            nc.vector.tensor_scalar(
                                out=P2h[:, jt, :], in0=P_sb[:, jt, h, 0:33],
                                scalar1=e2f[:, jt, h:h + 1], scalar2=None,
                                op0=Alu.mult)
                            nc.scalar.activation(
                                out=r1[:, jt, :], in_=gb, func=Act.Relu,
                                bias=ne2[:, jt, h:h + 1], scale=f2f[:, jt, h:h + 1])
                        v = uvpool.tile([128, ST, S], f16, tag="uv")
                        eng = nc.gpsimd if mask_on_pool else nc.vector
                        eng.tensor_tensor(
                            out=v[:].rearrange("p j (s i) -> p j s i", i=128),
                            in0=r1[:].rearrange("p j (s i) -> p j s i", i=128),
                            in1=ktg[:].rearrange("p s j i -> p j s i"),
                            op=Alu.mult)
                        for ib in range(ST):
                            for jt in range(ST):
                                nc.tensor.matmul(
                                    pav[:, ib, 0:33],
                                    ktg[:, ib, jt, :],
                                    P2h[:, jt, :],
                                    start=(jt == 0), stop=False)
                            for jt in range(ST):
                                nc.tensor.matmul(
                                    pav[:, ib, 0:33],
                                    v[:, jt, ib * 128:(ib + 1) * 128],
                                    P_sb[:, jt, h, 0:33],
                                    start=False, stop=(jt == ST - 1))
                    else:
                        # DVE path: u = max(g_i * f2_j, e2_j), v = u * K
                        u = uvpool.tile([128, ST, S], f16, tag="uv")
                        for jt in range(ST):
                            nc.vector.tensor_scalar(
                                out=u[:, jt, :], in0=gb,
                                scalar1=f2f[:, jt, h:h + 1],
                                scalar2=e2f[:, jt, h:h + 1],
                                op0=Alu.mult, op1=Alu.max)
                        v = uvpool.tile([128, ST, S], f16, tag="uv")
                        eng = nc.gpsimd if mask_on_pool else nc.vector
                        eng.tensor_tensor(
                            out=v[:].rearrange("p j (s i) -> p j s i", i=128),
                            in0=u[:].rearrange("p j (s i) -> p j s i", i=128),
                            in1=ktg[:].rearrange("p s j i -> p j s i"),
                            op=Alu.mult)
                        for ib in range(ST):
                            for jt in range(ST):
                                nc.tensor.matmul(
                                    pav[:, ib, 0:33],
                                    v[:, jt, ib * 128:(ib + 1) * 128],
                                    P_sb[:, jt, h, 0:33],
                                    start=(jt == 0), stop=(jt == ST - 1))
                    rec = small.tile([128, ST], f32, tag="rec")
                    nc.vector.reciprocal(out=rec[:], in_=pav[:, :, 32])
                    nc.vector.tensor_tensor(
                        out=conc[:, :, h * DK:(h + 1) * DK],
                        in0=pav[:, :, 0:DK],
                        in1=rec[:].rearrange("p (s one) -> p s one", one=1).broadcast_to([128, ST, DK]),
                        op=Alu.mult)

                # Residual + LayerNorm
                rr = lay.tile([128, ST, M], f16, tag="rr")
                sums = small.tile([128, ST], f32, tag="sums")
                sq = small.tile([128, ST], f32, tag="sq")
                for s in range(ST):
                    nc.vector.scalar_tensor_tensor(
                        out=rr[:, s, :], in0=conc[:, s, :], scalar=0.0, in1=x16[:, s, :],
                        op0=Alu.add, op1=Alu.add, accum_out=sums[:, s:s + 1])
                    scr = small.tile([128, M], f32, tag="scr")
                    nc.scalar.activation(out=scr[:], in_=rr[:, s, :], func=Act.Square,
                                         accum_out=sq[:, s:s + 1])
                mu = small.tile([128, ST], f32, tag="mu")
                nc.vector.tensor_scalar(out=mu[:], in0=sums[:], scalar1=1.0 / M,
                                        scalar2=None, op0=Alu.mult)
                mu2 = small.tile([128, ST], f32, tag="mu2")
                nc.vector.tensor_tensor(out=mu2[:], in0=mu[:], in1=mu[:], op=Alu.mult)
                var = small.tile([128, ST], f32, tag="var")
                nc.vector.scalar_tensor_tensor(
                    out=var[:], in0=sq[:], scalar=1.0 / M, in1=mu2[:],
                    op0=Alu.mult, op1=Alu.subtract)
                # rstd = 1/sqrt(var+eps) via Babylonian iterations + reciprocal
                # (avoids Sqrt/Ln ACT table switches away from the exp set)
                ve = small.tile([128, ST], f32, tag="ve")
                nc.vector.tensor_scalar(out=ve[:], in0=var[:], scalar1=LN_EPS,
                                        scalar2=None, op0=Alu.add)
                std = small.tile([128, ST], f32, tag="std")
                nc.vector.tensor_scalar(out=std[:], in0=ve[:], scalar1=0.4,
                                        scalar2=0.7, op0=Alu.mult, op1=Alu.add)
                for _it in range(3):
                    rs = small.tile([128, ST], f32, tag="rs", name=f"rs{_it}")
                    nc.vector.reciprocal(out=rs[:], in_=std[:])
                    tdiv = small.tile([128, ST], f32, tag="tdiv", name=f"tdiv{_it}")
                    nc.vector.tensor_tensor(out=tdiv[:], in0=ve[:], in1=rs[:],
                                            op=Alu.mult)
                    usum = small.tile([128, ST], f32, tag="usum", name=f"usum{_it}")
                    nc.vector.tensor_tensor(out=usum[:], in0=std[:], in1=tdiv[:],
                                            op=Alu.add)
                    std2 = small.tile([128, ST], f32, tag="std", name=f"std{_it}")
                    nc.vector.tensor_scalar(out=std2[:], in0=usum[:], scalar1=0.5,
                                            scalar2=None, op0=Alu.mult)
                    std = std2
                rstd = small.tile([128, ST], f32, tag="rstd")
                nc.vector.reciprocal(out=rstd[:], in_=std[:])

                last = (l == L - 1)
                if last:
                    y32 = ypool.tile([128, ST, M], f32, tag="y32")
                else:
                    xf16[(b, l + 1)] = xpool.tile([128, ST, M], f16, tag="xf16", name=f"xf16_{b}_{l+1}")
                for s in range(ST):
                    if apply_g:
                        tmp = small.tile([128, M], f32, tag="ytmp")
                        nc.vector.tensor_scalar(
                            out=tmp[:], in0=rr[:, s, :], scalar1=mu[:, s:s + 1],
                            scalar2=rstd[:, s:s + 1], op0=Alu.subtract, op1=Alu.mult)
                        tmp2 = small.tile([128, M], f32, tag="ytmp2")
                        nc.vector.tensor_tensor(out=tmp2[:], in0=tmp[:], in1=gb_ln[:], op=Alu.mult)
                        ydst = y32[:, s, :] if last else xf16[(b, l + 1)][:, s, :]
                        nc.vector.tensor_tensor(out=ydst, in0=tmp2[:], in1=bb_ln[:], op=Alu.add)
                    else:
                        ydst = y32[:, s, :] if last else xf16[(b, l + 1)][:, s, :]
                        nc.vector.tensor_scalar(
                            out=ydst, in0=rr[:, s, :], scalar1=mu[:, s:s + 1],
                            scalar2=rstd[:, s:s + 1], op0=Alu.subtract, op1=Alu.mult)
                if last:
                    nc.sync.dma_start(
                        out=out_d[b].rearrange("(s p) m -> p s m", p=128), in_=y32[:])
    nc.compile()
    return nc


def _get_nc(key):
    if key not in _CACHE:
        _CACHE[key] = _build(*key)
    return _CACHE[key]


def _pack_weights(proj_w, proj_b, attn_w, attn_b):
    L, H, M, DK = proj_w.shape
    KT = M // 128
    HC = H * 35
    pwcat = np.zeros((L, M, H, 35), np.float32)
    biascat = np.zeros((L, H, 35), np.float32)
    for l in range(L):
        a1, a2 = attn_w[l, :DK], attn_w[l, DK:]
        for h in range(H):
            pwcat[l, :, h, :32] = proj_w[l, h]
            pwcat[l, :, h, 33] = proj_w[l, h] @ a2
            pwcat[l, :, h, 34] = proj_w[l, h] @ a1
            biascat[l, h, :32] = proj_b[l, h]
            biascat[l, h, 32] = 1.0
            biascat[l, h, 33] = proj_b[l, h] @ a2
            biascat[l, h, 34] = proj_b[l, h] @ a1 + attn_b[l]
    return (pwcat.reshape(L, KT, 128, HC).astype(np.float16),
            biascat.reshape(L, HC))


def _prepare(adj, inputs, score_mask, type, proj_w, proj_b, attn_w, attn_b, ln_g, ln_b):
    adj = np.asarray(adj)
    inputs = np.asarray(inputs, dtype=np.float32)
    score_mask = np.asarray(score_mask)
    proj_w = np.asarray(proj_w, dtype=np.float32)
    proj_b = np.asarray(proj_b, dtype=np.float32)
    attn_w = np.asarray(attn_w, dtype=np.float32)
    attn_b = np.asarray(attn_b, dtype=np.float32)
    ln_g = np.asarray(ln_g, dtype=np.float32)
    ln_b = np.asarray(ln_b, dtype=np.float32)

    B, S, M = inputs.shape
    L, H = proj_w.shape[0], proj_w.shape[1]
    NCORES = 8
    B2 = B // NCORES
    semantic = bool(np.asarray(type) == 1)
    apply_g = not (np.allclose(ln_g, 1.0) and np.allclose(ln_b, 0.0))

    pwcat, biascat = _pack_weights(proj_w, proj_b, attn_w, attn_b)
    sm_u8 = np.ascontiguousarray(score_mask[:, 0]).astype(np.uint8)
    adj_i32 = np.ascontiguousarray(adj.astype(np.int32))

    in_maps = []
    for c in range(NCORES):
        m = {
            "adj": adj_i32[c * B2:(c + 1) * B2],
            "smask": sm_u8[c * B2:(c + 1) * B2],
            "x0": np.ascontiguousarray(inputs[c * B2:(c + 1) * B2]),
            "pwcat": pwcat, "biascat": biascat,
        }
        if apply_g:
            m["lng"] = ln_g
            m["lnb"] = ln_b
        in_maps.append(m)

    return (B2, S, M, H, L, semantic, apply_g), in_maps


def kernel(**inputs):
    from concourse.bass_utils import run_bass_kernel_spmd
    key, in_maps = _prepare(**inputs)
    nc = _get_nc(key)
    res = run_bass_kernel_spmd(nc, in_maps, core_ids=list(range(len(in_maps))),
                               trace=bool(int(os.environ.get("GAT_TRACE", "0"))))
    global LAST_EXEC_NS
    LAST_EXEC_NS = res.exec_time_ns
    out = np.concatenate([r["out"] for r in res.results], axis=0)
    return out.astype(np.float32)


def measure_hw_s(reps=64, n_runs=3, **inputs):
    """Estimate per-iteration device time by timing a reps-looped variant
    against the reps=1 variant (amortizes axon dispatch + transfer)."""
    import time
    from concourse.bass_utils import run_bass_kernel_spmd
    key, in_maps = _prepare(**inputs)
    cores = list(range(len(in_maps)))
    nc1 = _get_nc(key)
    ncR = _build(*key, reps=reps)

    def timed(nc):
        best = None
        for _ in range(n_runs):
            t0 = time.time()
            run_bass_kernel_spmd(nc, in_maps, core_ids=cores)
            dt = time.time() - t0
            best = dt if best is None else min(best, dt)
        return best

    t1 = timed(nc1)
    tR = timed(ncR)
    per_iter = (tR - t1) / (reps - 1)
    return per_iter, t1, tR
